# revision 3
# baseline (speedup 1.0000x reference)
"""Trainium2 Bass kernel for the KSubspaceBaseModel objective.

Reference computes, for B=2048 samples x (B, D=1024) and subspace bases
Us (R=4, K=16, D, d=32):
    z = x @ U; x_ = z @ U^T; loss = 0.5*||x - x_||^2  (per b, r, k)
    obj_r = mean_b min_k loss

Algebraic collapse: with G = U^T U and L = chol(I - 0.5 G) folded host-side
(Ut = U @ L), loss = 0.5||x||^2 - ||Ut^T x||^2, so the device computes
z~ = Ut^T x, squares, sums each subspace's 32 latent columns, takes max_k.
The 0.5||x||^2 base term (0.05% of the FLOPs) is summed host-side in fp64.

Device speed notes:
  * fp8 e4m3 operands (Ut scaled by 4096 to stay in the normal range) with
    DoubleRow matmuls: each instruction consumes TWO 128-deep contraction
    chunks ([128, 2, M] stationary x [128, 2, N] moving), 2x PE throughput
    and half the DMA bytes.
  * one PSUM bank per output group (8 groups of 512 cols) so no matmul ever
    waits on an epilogue reader (WAR-free stream); the PE p-state ramp
    (0.65 -> 1.2 -> 2.4 GHz after 3us continuous) then works for us.
  * fp8 warm-up matmuls bridge the DMA head so the ramp clock starts early
    and the stream stays gap-free.
  * group-major matmul order (a group = 512 output cols = one PSUM bank,
    accumulated over 4 kc-pairs) so each group's epilogue (ScalarE square,
    DVE subspace-sum + k-max) overlaps the next group's matmuls.
  * 3 parallel input DGE rings (sync=u0, scalar=xt, gpsimd SWDGE=u1);
    output staged as two DMAs so most of it leaves before the last group.
"""

import numpy as np
import ml_dtypes

import concourse.bass as bass
import concourse.bacc as bacc
import concourse.mybir as mybir
import concourse.tile as tile
from concourse.bass_utils import run_bass_kernel_spmd

B, D, R, K, d = 2048, 1024, 4, 16, 32
NCORES = 8
NB = B // 4          # 512 samples per core
BC = NB // 128       # 4 batch chunks per core
NJ = 4               # contraction pairs (8 kc chunks, 2 per DoubleRow matmul)
SCALE = 4096.0       # Ut pre-scale so fp8 e4m3 values are normal-range
WARM = 12            # PE warm-up matmuls (p-state ramp during DMA head)

FP8 = mybir.dt.float8e4
BF16 = mybir.dt.bfloat16
FP32 = mybir.dt.float32

_COMPILED = {}
LAST_RESULTS = None


def _build():
    nc = bacc.Bacc("TRN2", target_bir_lowering=False, debug=False)
    # host pre-arranges every tensor into its exact SBUF image so each
    # partition's DMA read is one contiguous run
    xt = nc.dram_tensor("xt", [128, BC * NJ * 2 * 128], FP8,
                        kind="ExternalInput")        # [p, bc, j, i, b]
    u0 = nc.dram_tensor("u0", [128, NJ * 2 * 512], FP8, kind="ExternalInput")
    u1 = nc.dram_tensor("u1", [128, NJ * 2 * 512], FP8, kind="ExternalInput")
    outp = nc.dram_tensor("outp", [128, 8], FP32, kind="ExternalOutput")

    xt_v = xt.ap().rearrange("p (b j i n) -> p b j i n", b=BC, j=NJ, i=2)
    u_v = [u.ap().rearrange("p (j i c) -> p j i c", j=NJ, i=2)
           for u in (u0, u1)]
    DR = mybir.MatmulPerfMode.DoubleRow

    with tile.TileContext(nc) as tc:
        with (
            tc.tile_pool(name="xsb", bufs=1) as xpool,
            tc.tile_pool(name="usb", bufs=1) as upool,
            tc.tile_pool(name="esb", bufs=4) as epool,
            tc.tile_pool(name="asb", bufs=2) as apool,
            tc.tile_pool(name="single", bufs=1) as spool,
            tc.tile_pool(name="zp", bufs=1, space="PSUM") as zpool,
        ):
            # per-chunk tiles so the first matmul only waits on its own
            # chunk's DMA, not the whole tensor (priority-ordered rings:
            # sync=u0 by pair, scalar=x by batch chunk, gpsimd SWDGE=u1)
            xb = [xpool.tile([128, NJ, 2, 128], FP8, tag=f"x{bc}",
                             name=f"x{bc}") for bc in range(BC)]
            uj = [[upool.tile([128, 2, 512], FP8, tag=f"u{nh}_{j}",
                              name=f"u{nh}_{j}") for j in range(NJ)]
                  for nh in range(2)]

            warm = spool.tile([128, 2, 384], FP8, tag="warm")
            nc.gpsimd.memset(warm[:], 0.0)

            for j in range(NJ):
                nc.sync.dma_start(uj[0][j][:], u_v[0][:, j])
            for bc in range(BC):
                nc.scalar.dma_start(xb[bc][:], xt_v[:, bc])
            for j in range(NJ):
                nc.gpsimd.dma_start(uj[1][j][:], u_v[1][:, j])

            # cols = per-group k-maxes in emission order (nh outer, bc inner)
            ostage_a = spool.tile([128, 6], FP32, tag="oa", name="ostage_a")
            ostage_b = spool.tile([128, 2], FP32, tag="ob", name="ostage_b")

            # one PSUM bank per group: matmuls never wait on a reader
            zps = {(nh, bc): zpool.tile([128, 512], FP32, tag=f"zp{nh}{bc}",
                                        name=f"zp{nh}{bc}")
                   for nh in range(2) for bc in range(BC)}

            # warm-up: keep the PE busy while the first chunks stream in so
            # the p-state ramp clock starts early (writes bank (0,0); the
            # real group 0 resets it with start=True)
            for _ in range(WARM):
                nc.tensor.matmul(zps[(0, 0)][:, 0:256], warm[:, :, 0:128],
                                 warm[:, :, 128:384], start=True, stop=True,
                                 perf_mode=DR, skip_group_check=True)

            gi = 0
            for nh in range(2):
                for bc in range(BC):
                    # moving operand is HW-capped at 512 streamed columns,
                    # so each group runs as two serial 256-col accumulations
                    e = epool.tile([128, 512], BF16, tag="e")
                    for half in range(2):
                        sl = slice(half * 256, (half + 1) * 256)
                        for j in range(NJ):
                            nc.tensor.matmul(
                                zps[(nh, bc)][:, sl], xb[bc][:, j],
                                uj[nh][j][:, :, sl],
                                start=(j == 0), stop=(j == NJ - 1),
                                perf_mode=DR, skip_group_check=True)
                        nc.scalar.square(e[:, sl], zps[(nh, bc)][:, sl])
                    a = apool.tile([128, K], FP32, tag="a")
                    nc.vector.reduce_sum(
                        a[:], e.rearrange("p (k c) -> p k c", c=d),
                        axis=mybir.AxisListType.X)
                    col = gi
                    dst, dcol = (ostage_a, col) if col < 6 else \
                                (ostage_b, col - 6)
                    nc.vector.reduce_max(dst[:, dcol:dcol + 1], a[:],
                                         axis=mybir.AxisListType.X)
                    if gi == 5:
                        nc.sync.dma_start(outp.ap()[:, 0:6], ostage_a[:])
                    gi += 1
            nc.sync.dma_start(outp.ap()[:, 6:8], ostage_b[:])

    nc.compile()
    return nc


def _prep(x, Us):
    # fold chol(I - 0.5 U^T U) into U, then scale+quantize to fp8 e4m3
    Us64 = Us.astype(np.float64)
    G = np.einsum('skDa,skDb->skab', Us64, Us64)
    L = np.linalg.cholesky(np.eye(d)[None, None] - 0.5 * G)
    Ut = np.einsum('skDa,skab->skDb', Us64, L)                # (R,K,D,d)
    u8 = (Ut * SCALE).astype(np.float32).astype(ml_dtypes.float8_e4m3)
    x8 = np.ascontiguousarray(x.T).astype(ml_dtypes.float8_e4m3)  # (D, B)

    def u_img(r):  # one replicate -> [128, NJ*2*512] (p, j, i, c)
        ur = np.ascontiguousarray(u8[r].transpose(1, 0, 2)).reshape(D, K * d)
        return np.ascontiguousarray(
            ur.reshape(NJ, 2, 128, K * d).transpose(2, 0, 1, 3)
        ).reshape(128, NJ * 2 * K * d)

    def x_img(b4):  # one batch quarter -> [128, BC*NJ*2*128] (p, bc, j, i, n)
        xc = x8[:, NB * b4: NB * (b4 + 1)]                    # (D, 512)
        return np.ascontiguousarray(
            xc.reshape(NJ, 2, 128, BC, 128).transpose(2, 3, 0, 1, 4)
        ).reshape(128, BC * NJ * 2 * 128)

    u_imgs = [u_img(r) for r in range(R)]
    x_imgs = [x_img(b4) for b4 in range(BC)]
    in_maps = []
    for c in range(NCORES):
        s2, b4 = c // 4, c % 4
        in_maps.append({
            "xt": x_imgs[b4],
            "u0": u_imgs[2 * s2],
            "u1": u_imgs[2 * s2 + 1],
        })
    return in_maps


def kernel(x, Us, _trace=False):
    global LAST_RESULTS
    x = np.asarray(x)
    Us = np.asarray(Us)
    if "nc" not in _COMPILED:
        _COMPILED["nc"] = _build()
    nc = _COMPILED["nc"]
    in_maps = _prep(x, Us)
    res = run_bass_kernel_spmd(nc, in_maps, core_ids=list(range(NCORES)),
                               trace=_trace)
    LAST_RESULTS = res
    # base term: exact fp64 sum of squares on host (tiny vs the device work)
    base = 0.5 * float(np.sum(x.astype(np.float64) ** 2)) / B
    obj = np.empty(R, np.float32)
    for r in range(R):
        s2, nh = r // 2, r % 2
        # group col = nh*4 + bc; z~ was scaled by SCALE
        cols = [res.results[4 * s2 + b4]["outp"][:, nh * 4 + bc]
                for b4 in range(4) for bc in range(BC)]
        term = np.mean(np.stack(cols).astype(np.float64)) / (SCALE * SCALE)
        obj[r] = np.float32(base - term)
    return obj


# revision 5
# speedup vs baseline: 1.0109x; 1.0109x over previous
"""Trainium2 Bass kernel for the KSubspaceBaseModel objective.

Reference computes, for B=2048 samples x (B, D=1024) and subspace bases
Us (R=4, K=16, D, d=32):
    z = x @ U; x_ = z @ U^T; loss = 0.5*||x - x_||^2  (per b, r, k)
    obj_r = mean_b min_k loss

Algebraic collapse: with G = U^T U and L = chol(I - 0.5 G) folded host-side
(Ut = U @ L), loss = 0.5||x||^2 - ||Ut^T x||^2, so the device computes
z~ = Ut^T x, squares, and sums each subspace's 32 latent columns; the
per-k sums ship out and the host takes max_k and the fp64 base term.

Device speed notes:
  * fp8 e4m3 operands (Ut scaled by 4096 to stay in the normal range) with
    DoubleRow matmuls: each instruction consumes TWO 128-deep contraction
    chunks ([128, 2, M] stationary x [128, 2, N] moving), 2x PE throughput
    and half the DMA bytes.
  * COLUMN-major u chunking: each 256 KB u transfer carries all 4
    contraction pairs for a 256-column block, so a (replicate-half,
    batch-chunk, col-block) mini-group is fully computable as soon as its
    one u chunk + one x chunk land; epilogue work spreads across the DMA
    window instead of crowding after the last transfer.
  * one PSUM bank per (nh, bc) group: matmuls never wait on an epilogue
    reader, keeping the stream gap-free so the PE p-state ramp
    (0.65 -> 1.2 -> 2.4 GHz after 3us continuous) works for us; fp8
    warm-up matmuls bridge the DMA head.
  * per-k sums go to a [128, 128] staging tile written slice-wise by DVE
    and leave as two staged DMAs; k-max and batch-mean happen on host.
  * 3 parallel input DGE rings: sync = u[*][cblk0], gpsimd SWDGE =
    u[*][cblk1], scalar = x chunks (then ScalarE squares).
"""

import numpy as np
import ml_dtypes

import concourse.bass as bass
import concourse.bacc as bacc
import concourse.mybir as mybir
import concourse.tile as tile
from concourse.bass_utils import run_bass_kernel_spmd

B, D, R, K, d = 2048, 1024, 4, 16, 32
NCORES = 8
NB = B // 4          # 512 samples per core
BC = NB // 128       # 4 batch chunks per core
NJ = 4               # contraction pairs (8 kc chunks, 2 per DoubleRow matmul)
SCALE = 4096.0       # Ut pre-scale so fp8 e4m3 values are normal-range
WARM = 10            # PE warm-up matmuls (p-state ramp during DMA head)

FP8 = mybir.dt.float8e4
BF16 = mybir.dt.bfloat16
FP32 = mybir.dt.float32

_COMPILED = {}
LAST_RESULTS = None


def _build():
    nc = bacc.Bacc("TRN2", target_bir_lowering=False, debug=False)
    # host pre-arranges every tensor into its exact SBUF image so each
    # partition's DMA read is one contiguous run
    xt = nc.dram_tensor("xt", [128, BC * NJ * 2 * 128], FP8,
                        kind="ExternalInput")        # [p, bc, j, i, b]
    u0 = nc.dram_tensor("u0", [128, 2 * NJ * 2 * 256], FP8,
                        kind="ExternalInput")        # [p, cb, j, i, c]
    u1 = nc.dram_tensor("u1", [128, 2 * NJ * 2 * 256], FP8,
                        kind="ExternalInput")
    outp = nc.dram_tensor("outp", [128, 128], FP32, kind="ExternalOutput")

    xt_v = xt.ap().rearrange("p (b j i n) -> p b j i n", b=BC, j=NJ, i=2)
    u_v = [u.ap().rearrange("p (cb j i c) -> p cb j i c", cb=2, j=NJ, i=2)
           for u in (u0, u1)]
    DR = mybir.MatmulPerfMode.DoubleRow

    with tile.TileContext(nc) as tc:
        with (
            tc.tile_pool(name="xsb", bufs=1) as xpool,
            tc.tile_pool(name="usb", bufs=1) as upool,
            tc.tile_pool(name="esb", bufs=6) as epool,
            tc.tile_pool(name="single", bufs=1) as spool,
            tc.tile_pool(name="zp", bufs=1, space="PSUM") as zpool,
        ):
            # per-chunk tiles so a matmul only waits on its own chunk's DMA
            xb = [xpool.tile([128, NJ, 2, 128], FP8, tag=f"x{bc}",
                             name=f"x{bc}") for bc in range(BC)]
            uc = {(nh, cb): upool.tile([128, NJ, 2, 256], FP8,
                                       tag=f"u{nh}_{cb}", name=f"u{nh}_{cb}")
                  for nh in range(2) for cb in range(2)}

            warm = spool.tile([128, 2, 384], FP8, tag="warm")
            nc.gpsimd.memset(warm[:], 0.0)

            # rings: sync = cblk0 of both replicates, SWDGE = cblk1,
            # scalar = x (and afterwards only ScalarE squares)
            nc.sync.dma_start(uc[(0, 0)][:], u_v[0][:, 0])
            nc.sync.dma_start(uc[(1, 0)][:], u_v[1][:, 0])
            for bc in range(BC):
                nc.scalar.dma_start(xb[bc][:], xt_v[:, bc])
            nc.gpsimd.dma_start(uc[(0, 1)][:], u_v[0][:, 1])
            nc.gpsimd.dma_start(uc[(1, 1)][:], u_v[1][:, 1])

            # per-k sums land here slice-wise: group g = nh*4+bc covers
            # cols [g*16, (g+1)*16)
            ostage = spool.tile([128, 128], FP32, tag="os", name="ostage")

            # one PSUM bank per group: matmuls never wait on a reader
            zps = {(nh, bc): zpool.tile([128, 512], FP32, tag=f"zp{nh}{bc}",
                                        name=f"zp{nh}{bc}")
                   for nh in range(2) for bc in range(BC)}

            # warm-up: keep the PE busy while the first chunks stream in so
            # the p-state ramp clock starts early (writes bank (0,0); the
            # real group 0 resets it with start=True)
            for _ in range(WARM):
                nc.tensor.matmul(zps[(0, 0)][:, 0:256], warm[:, :, 0:128],
                                 warm[:, :, 128:384], start=True, stop=True,
                                 perf_mode=DR, skip_group_check=True)

            et = {}
            for nh in range(2):
                for bc in range(BC):
                    et[(nh, bc)] = epool.tile([128, 512], BF16, tag="e",
                                              name=f"e{nh}{bc}")
            for nh in range(2):
                for cb in range(2):
                    sl = slice(cb * 256, (cb + 1) * 256)
                    for bc in range(BC):
                        for j in range(NJ):
                            nc.tensor.matmul(
                                zps[(nh, bc)][:, sl], xb[bc][:, j],
                                uc[(nh, cb)][:, j],
                                start=(j == 0), stop=(j == NJ - 1),
                                perf_mode=DR, skip_group_check=True)
                        nc.scalar.square(et[(nh, bc)][:, sl],
                                         zps[(nh, bc)][:, sl])
                        if cb == 1:
                            g = nh * 4 + bc
                            nc.vector.reduce_sum(
                                ostage[:, g * K:(g + 1) * K],
                                et[(nh, bc)].rearrange(
                                    "p (k c) -> p k c", c=d),
                                axis=mybir.AxisListType.X)
                            if g == 5:
                                nc.sync.dma_start(outp.ap()[:, 0:96],
                                                  ostage[:, 0:96])
            nc.sync.dma_start(outp.ap()[:, 96:128], ostage[:, 96:128])

    nc.compile()
    return nc


def _prep(x, Us):
    # fold chol(I - 0.5 U^T U) into U, then scale+quantize to fp8 e4m3
    Us64 = Us.astype(np.float64)
    G = np.einsum('skDa,skDb->skab', Us64, Us64)
    L = np.linalg.cholesky(np.eye(d)[None, None] - 0.5 * G)
    Ut = np.einsum('skDa,skab->skDb', Us64, L)                # (R,K,D,d)
    u8 = (Ut * SCALE).astype(np.float32).astype(ml_dtypes.float8_e4m3)
    x8 = np.ascontiguousarray(x.T).astype(ml_dtypes.float8_e4m3)  # (D, B)

    def u_img(r):  # one replicate -> [128, 2*NJ*2*256] (p, cb, j, i, c)
        ur = np.ascontiguousarray(u8[r].transpose(1, 0, 2)).reshape(D, K * d)
        return np.ascontiguousarray(
            ur.reshape(NJ, 2, 128, 2, 256).transpose(2, 3, 0, 1, 4)
        ).reshape(128, 2 * NJ * 2 * 256)

    def x_img(b4):  # one batch quarter -> [128, BC*NJ*2*128] (p, bc, j, i, n)
        xc = x8[:, NB * b4: NB * (b4 + 1)]                    # (D, 512)
        return np.ascontiguousarray(
            xc.reshape(NJ, 2, 128, BC, 128).transpose(2, 3, 0, 1, 4)
        ).reshape(128, BC * NJ * 2 * 128)

    u_imgs = [u_img(r) for r in range(R)]
    x_imgs = [x_img(b4) for b4 in range(BC)]
    in_maps = []
    for c in range(NCORES):
        s2, b4 = c // 4, c % 4
        in_maps.append({
            "xt": x_imgs[b4],
            "u0": u_imgs[2 * s2],
            "u1": u_imgs[2 * s2 + 1],
        })
    return in_maps


def kernel(x, Us, _trace=False):
    global LAST_RESULTS
    x = np.asarray(x)
    Us = np.asarray(Us)
    if "nc" not in _COMPILED:
        _COMPILED["nc"] = _build()
    nc = _COMPILED["nc"]
    in_maps = _prep(x, Us)
    res = run_bass_kernel_spmd(nc, in_maps, core_ids=list(range(NCORES)),
                               trace=_trace)
    LAST_RESULTS = res
    # base term: exact fp64 sum of squares on host (tiny vs the device work)
    base = 0.5 * float(np.sum(x.astype(np.float64) ** 2)) / B
    obj = np.empty(R, np.float32)
    for r in range(R):
        s2, nh = r // 2, r % 2
        # group col block = (nh*4 + bc)*16; z~ was scaled by SCALE
        terms = []
        for b4 in range(4):
            o = res.results[4 * s2 + b4]["outp"]              # [128, 128]
            for bc in range(BC):
                g = nh * 4 + bc
                terms.append(o[:, g * K:(g + 1) * K].astype(np.float64)
                             .max(axis=1))
        term = np.mean(np.stack(terms)) / (SCALE * SCALE)
        obj[r] = np.float32(base - term)
    return obj


# revision 7
# speedup vs baseline: 1.0234x; 1.0124x over previous
"""Trainium2 Bass kernel for the KSubspaceBaseModel objective.

Reference computes, for B=2048 samples x (B, D=1024) and subspace bases
Us (R=4, K=16, D, d=32):
    z = x @ U; x_ = z @ U^T; loss = 0.5*||x - x_||^2  (per b, r, k)
    obj_r = mean_b min_k loss

Algebraic collapse: with G = U^T U and L = chol(I - 0.5 G) folded host-side
(Ut = U @ L), loss = 0.5||x||^2 - ||Ut^T x||^2, so the device computes
z~ = Ut^T x, squares, and sums each subspace's 32 latent columns; the
per-k sums ship out and the host takes max_k and the fp64 base term.

Device speed notes:
  * fp8 e4m3 operands (Ut scaled by 4096 to stay in the normal range) with
    DoubleRow matmuls: each instruction consumes TWO 128-deep contraction
    chunks ([128, 2, M] stationary x [128, 2, N] moving), 2x PE throughput
    and half the DMA bytes.
  * COLUMN-major u chunking in CONSUMPTION order: each 256 KB u transfer
    carries all 4 contraction pairs for a 256-column block, and the sync
    ring's FIFO [u0c0, u0c1, u1c0, u1c1] means every chunk that lands
    unlocks 16 matmuls immediately (DMA engines round-robin across rings,
    so in-flight transfers finish roughly in issue order).
  * one PSUM bank per (nh, bc) group: matmuls never wait on an epilogue
    reader, keeping the stream gap-free so the PE p-state ramp
    (0.65 -> 1.2 -> 2.4 GHz after a few us continuous) works for us; fp8
    warm-up matmuls bridge the DMA head.
  * epilogue spread over three engines: ScalarE squares (PSUM -> bf16),
    GpSimd sums replicate-0 groups, DVE sums replicate-1 groups per
    half-block so the last chunk's tail is short; k-max and batch-mean
    happen on host from the shipped [128, 8*16] per-k sums.
"""

import numpy as np
import ml_dtypes

import concourse.bass as bass
import concourse.bacc as bacc
import concourse.mybir as mybir
import concourse.tile as tile
from concourse.bass_utils import run_bass_kernel_spmd

B, D, R, K, d = 2048, 1024, 4, 16, 32
NCORES = 8
NB = B // 4          # 512 samples per core
BC = NB // 128       # 4 batch chunks per core
NJ = 4               # contraction pairs (8 kc chunks, 2 per DoubleRow matmul)
SCALE = 4096.0       # Ut pre-scale so fp8 e4m3 values are normal-range
WARM = 10            # PE warm-up matmuls (p-state ramp during DMA head)

FP8 = mybir.dt.float8e4
BF16 = mybir.dt.bfloat16
FP32 = mybir.dt.float32

_COMPILED = {}
LAST_RESULTS = None


def _build():
    nc = bacc.Bacc("TRN2", target_bir_lowering=False, debug=False)
    # host pre-arranges every tensor into its exact SBUF image so each
    # partition's DMA read is one contiguous run
    xt = nc.dram_tensor("xt", [128, BC * NJ * 2 * 128], FP8,
                        kind="ExternalInput")        # [p, bc, j, i, b]
    u0 = nc.dram_tensor("u0", [128, 2 * NJ * 2 * 256], FP8,
                        kind="ExternalInput")        # [p, cb, j, i, c]
    u1 = nc.dram_tensor("u1", [128, 2 * NJ * 2 * 256], FP8,
                        kind="ExternalInput")
    outp = nc.dram_tensor("outp", [128, 128], FP32, kind="ExternalOutput")

    xt_v = xt.ap().rearrange("p (b j i n) -> p b j i n", b=BC, j=NJ, i=2)
    u_v = [u.ap().rearrange("p (cb j i c) -> p cb j i c", cb=2, j=NJ, i=2)
           for u in (u0, u1)]
    DR = mybir.MatmulPerfMode.DoubleRow

    # phase order = chunk consumption order on the sync ring
    PHASES = [(0, 0), (0, 1), (1, 0), (1, 1)]

    with tile.TileContext(nc) as tc:
        with (
            tc.tile_pool(name="xsb", bufs=1) as xpool,
            tc.tile_pool(name="usb", bufs=1) as upool,
            tc.tile_pool(name="esb", bufs=1) as epool,
            tc.tile_pool(name="single", bufs=1) as spool,
            tc.tile_pool(name="zp", bufs=1, space="PSUM") as zpool,
        ):
            # per-chunk tiles so a matmul only waits on its own chunk's DMA
            xb = [xpool.tile([128, NJ, 2, 128], FP8, tag=f"x{bc}",
                             name=f"x{bc}") for bc in range(BC)]
            uc = {(nh, cb): upool.tile([128, NJ, 2, 256], FP8,
                                       tag=f"u{nh}_{cb}", name=f"u{nh}_{cb}")
                  for nh in range(2) for cb in range(2)}

            warm = spool.tile([128, 2, 384], FP8, tag="warm")
            nc.gpsimd.memset(warm[:], 0.0)

            # sync ring: u chunks in consumption order; scalar ring: x
            for nh, cb in PHASES:
                nc.sync.dma_start(uc[(nh, cb)][:], u_v[nh][:, cb])
            for bc in range(BC):
                nc.scalar.dma_start(xb[bc][:], xt_v[:, bc])

            # per-k sums land here slice-wise: group g = nh*4+bc covers
            # cols [g*16, (g+1)*16)
            ostage = spool.tile([128, 128], FP32, tag="os", name="ostage")

            # one PSUM bank per group: matmuls never wait on a reader
            zps = {(nh, bc): zpool.tile([128, 512], FP32, tag=f"zp{nh}{bc}",
                                        name=f"zp{nh}{bc}")
                   for nh in range(2) for bc in range(BC)}

            # warm-up: keep the PE busy while the first chunks stream in so
            # the p-state ramp clock starts early (writes bank (0,0); the
            # real group 0 resets it with start=True)
            for _ in range(WARM):
                nc.tensor.matmul(zps[(0, 0)][:, 0:256], warm[:, :, 0:128],
                                 warm[:, :, 128:384], start=True, stop=True,
                                 perf_mode=DR, skip_group_check=True)

            et = {(nh, bc): epool.tile([128, 512], BF16, tag=f"e{nh}{bc}",
                                       name=f"e{nh}{bc}")
                  for nh in range(2) for bc in range(BC)}

            for nh, cb in PHASES:
                sl = slice(cb * 256, (cb + 1) * 256)
                for bc in range(BC):
                    g = nh * 4 + bc
                    for j in range(NJ):
                        nc.tensor.matmul(
                            zps[(nh, bc)][:, sl], xb[bc][:, j],
                            uc[(nh, cb)][:, j],
                            start=(j == 0), stop=(j == NJ - 1),
                            perf_mode=DR, skip_group_check=True)
                    nc.scalar.square(et[(nh, bc)][:, sl],
                                     zps[(nh, bc)][:, sl])
                    if nh == 0 and cb == 1:
                        # replicate-0 groups complete here: one 512-col
                        # subspace-sum per group
                        nc.vector.reduce_sum(
                            ostage[:, g * K:(g + 1) * K],
                            et[(nh, bc)].rearrange("p (k c) -> p k c", c=d),
                            axis=mybir.AxisListType.X)
                        if bc == BC - 1:
                            nc.sync.dma_start(outp.ap()[:, 0:64],
                                              ostage[:, 0:64])
                    elif nh == 1:
                        # replicate-1: per-half sums on DVE right after each
                        # square so the final chunk's tail is short
                        nc.vector.reduce_sum(
                            ostage[:, g * K + cb * 8:g * K + cb * 8 + 8],
                            et[(nh, bc)][:, sl].rearrange(
                                "p (k c) -> p k c", c=d),
                            axis=mybir.AxisListType.X)
            nc.sync.dma_start(outp.ap()[:, 64:128], ostage[:, 64:128])

    nc.compile()
    return nc


def _prep(x, Us):
    # fold chol(I - 0.5 U^T U) into U, then scale+quantize to fp8 e4m3
    Us64 = Us.astype(np.float64)
    G = np.einsum('skDa,skDb->skab', Us64, Us64)
    L = np.linalg.cholesky(np.eye(d)[None, None] - 0.5 * G)
    Ut = np.einsum('skDa,skab->skDb', Us64, L)                # (R,K,D,d)
    u8 = (Ut * SCALE).astype(np.float32).astype(ml_dtypes.float8_e4m3)
    x8 = np.ascontiguousarray(x.T).astype(ml_dtypes.float8_e4m3)  # (D, B)

    def u_img(r):  # one replicate -> [128, 2*NJ*2*256] (p, cb, j, i, c)
        ur = np.ascontiguousarray(u8[r].transpose(1, 0, 2)).reshape(D, K * d)
        return np.ascontiguousarray(
            ur.reshape(NJ, 2, 128, 2, 256).transpose(2, 3, 0, 1, 4)
        ).reshape(128, 2 * NJ * 2 * 256)

    def x_img(b4):  # one batch quarter -> [128, BC*NJ*2*128] (p, bc, j, i, n)
        xc = x8[:, NB * b4: NB * (b4 + 1)]                    # (D, 512)
        return np.ascontiguousarray(
            xc.reshape(NJ, 2, 128, BC, 128).transpose(2, 3, 0, 1, 4)
        ).reshape(128, BC * NJ * 2 * 128)

    u_imgs = [u_img(r) for r in range(R)]
    x_imgs = [x_img(b4) for b4 in range(BC)]
    in_maps = []
    for c in range(NCORES):
        s2, b4 = c // 4, c % 4
        in_maps.append({
            "xt": x_imgs[b4],
            "u0": u_imgs[2 * s2],
            "u1": u_imgs[2 * s2 + 1],
        })
    return in_maps


def kernel(x, Us, _trace=False):
    global LAST_RESULTS
    x = np.asarray(x)
    Us = np.asarray(Us)
    if "nc" not in _COMPILED:
        _COMPILED["nc"] = _build()
    nc = _COMPILED["nc"]
    in_maps = _prep(x, Us)
    res = run_bass_kernel_spmd(nc, in_maps, core_ids=list(range(NCORES)),
                               trace=_trace)
    LAST_RESULTS = res
    # base term: exact fp64 sum of squares on host (tiny vs the device work)
    base = 0.5 * float(np.sum(x.astype(np.float64) ** 2)) / B
    obj = np.empty(R, np.float32)
    for r in range(R):
        s2, nh = r // 2, r % 2
        # group col block = (nh*4 + bc)*16; z~ was scaled by SCALE
        terms = []
        for b4 in range(4):
            o = res.results[4 * s2 + b4]["outp"]              # [128, 128]
            for bc in range(BC):
                g = nh * 4 + bc
                terms.append(o[:, g * K:(g + 1) * K].astype(np.float64)
                             .max(axis=1))
        term = np.mean(np.stack(terms)) / (SCALE * SCALE)
        obj[r] = np.float32(base - term)
    return obj


# revision 10
# speedup vs baseline: 1.0628x; 1.0386x over previous
"""Trainium2 Bass kernel for the KSubspaceBaseModel objective.

Reference computes, for B=2048 samples x (B, D=1024) and subspace bases
Us (R=4, K=16, D, d=32):
    z = x @ U; x_ = z @ U^T; loss = 0.5*||x - x_||^2  (per b, r, k)
    obj_r = mean_b min_k loss

Algebraic collapse: with G = U^T U and L = chol(I - 0.5 G) folded host-side
(Ut = U @ L), loss = 0.5||x||^2 - ||Ut^T x||^2, so the device computes
z~ = Ut^T x, squares, and sums each subspace's 32 latent columns; the
per-k sums ship out and the host takes max_k and the fp64 base term.

Device speed notes:
  * fp8 e4m3 operands (Ut scaled by 4096 to stay in the normal range) with
    DoubleRow matmuls: each instruction consumes TWO 128-deep contraction
    chunks ([128, 2, M] stationary x [128, 2, N] moving), 2x PE throughput
    and half the DMA bytes.
  * COLUMN-major u chunking in CONSUMPTION order: each 256 KB u transfer
    carries all 4 contraction pairs for a 256-column block, and the sync
    ring's FIFO [u0c0, u0c1, u1c0, u1c1] means every chunk that lands
    unlocks 16 matmuls immediately (DMA engines round-robin across rings,
    so in-flight transfers finish roughly in issue order).
  * one PSUM bank per (nh, bc) group: matmuls never wait on an epilogue
    reader, keeping the stream gap-free so the PE p-state ramp
    (0.65 -> 1.2 -> 2.4 GHz after a few us continuous) works for us; fp8
    warm-up matmuls bridge the DMA head.
  * epilogue spread over three engines: ScalarE squares (PSUM -> bf16),
    GpSimd sums replicate-0 groups, DVE sums replicate-1 groups per
    half-block so the last chunk's tail is short; k-max and batch-mean
    happen on host from the shipped [128, 8*16] per-k sums.
"""

import numpy as np
import ml_dtypes

import concourse.bass as bass
import concourse.bacc as bacc
import concourse.mybir as mybir
import concourse.tile as tile
from concourse.bass_utils import run_bass_kernel_spmd

B, D, R, K, d = 2048, 1024, 4, 16, 32
NCORES = 8
NB = B // 4          # 512 samples per core
BC = NB // 128       # 4 batch chunks per core
NJ = 4               # contraction pairs (8 kc chunks, 2 per DoubleRow matmul)
SCALE = 4096.0       # Ut pre-scale so fp8 e4m3 values are normal-range
WARM = 8             # PE warm-up matmuls (p-state ramp during DMA head)

FP8 = mybir.dt.float8e4
BF16 = mybir.dt.bfloat16
FP32 = mybir.dt.float32

_COMPILED = {}
LAST_RESULTS = None


def _build():
    nc = bacc.Bacc("TRN2", target_bir_lowering=False, debug=False)
    # host pre-arranges every tensor into its exact SBUF image so each
    # partition's DMA read is one contiguous run
    xt = nc.dram_tensor("xt", [128, BC * NJ * 2 * 128], FP8,
                        kind="ExternalInput")        # [p, bc, j, i, b]
    u0 = nc.dram_tensor("u0", [128, 2 * NJ * 2 * 256], FP8,
                        kind="ExternalInput")        # [p, cb, j, i, c]
    u1 = nc.dram_tensor("u1", [128, 2 * NJ * 2 * 256], FP8,
                        kind="ExternalInput")
    outp = nc.dram_tensor("outp", [128, 128], FP32, kind="ExternalOutput")

    xt_v = xt.ap().rearrange("p (b j i n) -> p b j i n", b=BC, j=NJ, i=2)
    u_v = [u.ap().rearrange("p (cb j i c) -> p cb j i c", cb=2, j=NJ, i=2)
           for u in (u0, u1)]
    DR = mybir.MatmulPerfMode.DoubleRow

    # phase order = chunk consumption order on the sync ring
    PHASES = [(0, 0), (0, 1), (1, 0), (1, 1)]

    with tile.TileContext(nc) as tc:
        with (
            tc.tile_pool(name="xsb", bufs=1) as xpool,
            tc.tile_pool(name="usb", bufs=1) as upool,
            tc.tile_pool(name="esb", bufs=1) as epool,
            tc.tile_pool(name="single", bufs=1) as spool,
            tc.tile_pool(name="zp", bufs=1, space="PSUM") as zpool,
        ):
            # per-chunk tiles so a matmul only waits on its own chunk's DMA;
            # the first and last u chunks are split j-wise (128 KB) so the
            # stream starts earlier and the final chunk gates only 8 matmuls
            xb = [xpool.tile([128, NJ, 2, 128], FP8, tag=f"x{bc}",
                             name=f"x{bc}") for bc in range(BC)]
            uc = {}
            for nh, cb in PHASES:
                if (nh, cb) in ((0, 0), (1, 1)):
                    uc[(nh, cb)] = [
                        upool.tile([128, 2, 2, 256], FP8,
                                   tag=f"u{nh}_{cb}{h}", name=f"u{nh}_{cb}{h}")
                        for h in range(2)]
                else:
                    uc[(nh, cb)] = upool.tile(
                        [128, NJ, 2, 256], FP8,
                        tag=f"u{nh}_{cb}", name=f"u{nh}_{cb}")

            def uview(nh, cb, j):
                t = uc[(nh, cb)]
                if isinstance(t, list):
                    return t[j // 2][:, j % 2]
                return t[:, j]

            warm = spool.tile([128, 2, 384], FP8, tag="warm")
            nc.gpsimd.memset(warm[:], 0.0)

            # sync ring: u chunks in consumption order; scalar ring: x
            for nh, cb in PHASES:
                if isinstance(uc[(nh, cb)], list):
                    nc.sync.dma_start(uc[(nh, cb)][0][:],
                                      u_v[nh][:, cb, 0:2])
                    nc.sync.dma_start(uc[(nh, cb)][1][:],
                                      u_v[nh][:, cb, 2:4])
                else:
                    nc.sync.dma_start(uc[(nh, cb)][:], u_v[nh][:, cb])
            for bc in range(BC):
                nc.scalar.dma_start(xb[bc][:], xt_v[:, bc])

            # per-k sums land here slice-wise: group g = nh*4+bc covers
            # cols [g*16, (g+1)*16)
            ostage = spool.tile([128, 128], FP32, tag="os", name="ostage")

            # one PSUM bank per group: matmuls never wait on a reader
            zps = {(nh, bc): zpool.tile([128, 512], FP32, tag=f"zp{nh}{bc}",
                                        name=f"zp{nh}{bc}")
                   for nh in range(2) for bc in range(BC)}

            # warm-up: keep the PE busy while the first chunks stream in so
            # the p-state ramp clock starts early (writes bank (0,0); the
            # real group 0 resets it with start=True)
            for _ in range(WARM):
                nc.tensor.matmul(zps[(0, 0)][:, 0:256], warm[:, :, 0:128],
                                 warm[:, :, 128:384], start=True, stop=True,
                                 perf_mode=DR, skip_group_check=True)

            et = {(nh, bc): epool.tile([128, 512], BF16, tag=f"e{nh}{bc}",
                                       name=f"e{nh}{bc}")
                  for nh in range(2) for bc in range(BC)}

            for nh, cb in PHASES:
                sl = slice(cb * 256, (cb + 1) * 256)
                for bc in range(BC):
                    g = nh * 4 + bc
                    for j in range(NJ):
                        nc.tensor.matmul(
                            zps[(nh, bc)][:, sl], xb[bc][:, j],
                            uview(nh, cb, j),
                            start=(j == 0), stop=(j == NJ - 1),
                            perf_mode=DR, skip_group_check=True)
                    nc.scalar.square(et[(nh, bc)][:, sl],
                                     zps[(nh, bc)][:, sl])
                    if nh == 0 and cb == 1:
                        # replicate-0 groups complete here: one 512-col
                        # subspace-sum per group
                        nc.vector.reduce_sum(
                            ostage[:, g * K:(g + 1) * K],
                            et[(nh, bc)].rearrange("p (k c) -> p k c", c=d),
                            axis=mybir.AxisListType.X)
                        if bc == BC - 1:
                            nc.sync.dma_start(outp.ap()[:, 0:64],
                                              ostage[:, 0:64])
                    elif nh == 1:
                        # replicate-1: per-half sums on DVE right after each
                        # square so the final chunk's tail is short
                        nc.vector.reduce_sum(
                            ostage[:, g * K + cb * 8:g * K + cb * 8 + 8],
                            et[(nh, bc)][:, sl].rearrange(
                                "p (k c) -> p k c", c=d),
                            axis=mybir.AxisListType.X)
            nc.sync.dma_start(outp.ap()[:, 64:128], ostage[:, 64:128])

    nc.compile()
    return nc


def _prep(x, Us):
    # fold chol(I - 0.5 U^T U) into U, then scale+quantize to fp8 e4m3
    Us64 = Us.astype(np.float64)
    G = np.einsum('skDa,skDb->skab', Us64, Us64)
    L = np.linalg.cholesky(np.eye(d)[None, None] - 0.5 * G)
    Ut = np.einsum('skDa,skab->skDb', Us64, L)                # (R,K,D,d)
    u8 = (Ut * SCALE).astype(np.float32).astype(ml_dtypes.float8_e4m3)
    x8 = np.ascontiguousarray(x.T).astype(ml_dtypes.float8_e4m3)  # (D, B)

    def u_img(r):  # one replicate -> [128, 2*NJ*2*256] (p, cb, j, i, c)
        ur = np.ascontiguousarray(u8[r].transpose(1, 0, 2)).reshape(D, K * d)
        return np.ascontiguousarray(
            ur.reshape(NJ, 2, 128, 2, 256).transpose(2, 3, 0, 1, 4)
        ).reshape(128, 2 * NJ * 2 * 256)

    def x_img(b4):  # one batch quarter -> [128, BC*NJ*2*128] (p, bc, j, i, n)
        xc = x8[:, NB * b4: NB * (b4 + 1)]                    # (D, 512)
        return np.ascontiguousarray(
            xc.reshape(NJ, 2, 128, BC, 128).transpose(2, 3, 0, 1, 4)
        ).reshape(128, BC * NJ * 2 * 128)

    u_imgs = [u_img(r) for r in range(R)]
    x_imgs = [x_img(b4) for b4 in range(BC)]
    in_maps = []
    for c in range(NCORES):
        s2, b4 = c // 4, c % 4
        in_maps.append({
            "xt": x_imgs[b4],
            "u0": u_imgs[2 * s2],
            "u1": u_imgs[2 * s2 + 1],
        })
    return in_maps


def kernel(x, Us, _trace=False):
    global LAST_RESULTS
    x = np.asarray(x)
    Us = np.asarray(Us)
    if "nc" not in _COMPILED:
        _COMPILED["nc"] = _build()
    nc = _COMPILED["nc"]
    in_maps = _prep(x, Us)
    res = run_bass_kernel_spmd(nc, in_maps, core_ids=list(range(NCORES)),
                               trace=_trace)
    LAST_RESULTS = res
    # base term: exact fp64 sum of squares on host (tiny vs the device work)
    base = 0.5 * float(np.sum(x.astype(np.float64) ** 2)) / B
    obj = np.empty(R, np.float32)
    for r in range(R):
        s2, nh = r // 2, r % 2
        # group col block = (nh*4 + bc)*16; z~ was scaled by SCALE
        terms = []
        for b4 in range(4):
            o = res.results[4 * s2 + b4]["outp"]              # [128, 128]
            for bc in range(BC):
                g = nh * 4 + bc
                terms.append(o[:, g * K:(g + 1) * K].astype(np.float64)
                             .max(axis=1))
        term = np.mean(np.stack(terms)) / (SCALE * SCALE)
        obj[r] = np.float32(base - term)
    return obj


# revision 14
# speedup vs baseline: 1.1775x; 1.1079x over previous
"""Trainium2 Bass kernel for the KSubspaceBaseModel objective.

Reference computes, for B=2048 samples x (B, D=1024) and subspace bases
Us (R=4, K=16, D, d=32):
    z = x @ U; x_ = z @ U^T; loss = 0.5*||x - x_||^2  (per b, r, k)
    obj_r = mean_b min_k loss

Algebraic collapse: with G = U^T U and L = chol(I - 0.5 G) folded host-side
(Ut = U @ L), loss = 0.5||x||^2 - ||Ut^T x||^2, so the device computes
z~ = Ut^T x, squares, and sums each subspace's 32 latent columns; the
per-k sums ship out and the host takes max_k and the fp64 base term.

Device speed notes:
  * fp8 e4m3 operands (Ut scaled by 4096 to stay in the normal range) with
    DoubleRow matmuls: each instruction consumes TWO 128-deep contraction
    chunks ([128, 2, M] stationary x [128, 2, N] moving), 2x PE throughput
    and half the DMA bytes.
  * COLUMN-major u chunking in CONSUMPTION order: each 256 KB u transfer
    carries all 4 contraction pairs for a 256-column block, and the sync
    ring's FIFO [u0c0, u0c1, u1c0, u1c1] means every chunk that lands
    unlocks 16 matmuls immediately (DMA engines round-robin across rings,
    so in-flight transfers finish roughly in issue order).
  * one PSUM bank per (nh, bc) group: matmuls never wait on an epilogue
    reader, keeping the stream gap-free so the PE p-state ramp
    (0.65 -> 1.2 -> 2.4 GHz after a few us continuous) works for us; fp8
    warm-up matmuls bridge the DMA head.
  * epilogue spread over three engines: ScalarE squares (PSUM -> bf16),
    GpSimd sums replicate-0 groups, DVE sums replicate-1 groups per
    half-block so the last chunk's tail is short; k-max and batch-mean
    happen on host from the shipped [128, 8*16] per-k sums.
"""

import numpy as np
import ml_dtypes

import concourse.bass as bass
import concourse.bacc as bacc
import concourse.mybir as mybir
import concourse.tile as tile
from concourse.bass_utils import run_bass_kernel_spmd

B, D, R, K, d = 2048, 1024, 4, 16, 32
NCORES = 8
NB = B // 4          # 512 samples per core
BC = NB // 128       # 4 batch chunks per core
NJ = 4               # contraction pairs (8 kc chunks, 2 per DoubleRow matmul)
SCALE = 4096.0       # Ut pre-scale so fp8 e4m3 values are normal-range
WARM = 12            # PE warm-up matmuls (p-state ramp during DMA head)

FP8 = mybir.dt.float8e4
BF16 = mybir.dt.bfloat16
FP32 = mybir.dt.float32

_COMPILED = {}
LAST_RESULTS = None


def _build():
    nc = bacc.Bacc("TRN2", target_bir_lowering=False, debug=False)
    # host pre-arranges every tensor into its exact SBUF image so each
    # partition's DMA read is one contiguous run
    xt = nc.dram_tensor("xt", [128, BC * NJ * 2 * 128], FP8,
                        kind="ExternalInput")        # [p, bc, j, i, b]
    u0 = nc.dram_tensor("u0", [128, 2 * NJ * 2 * 256], FP8,
                        kind="ExternalInput")        # [p, cb, j, i, c]
    u1 = nc.dram_tensor("u1", [128, 2 * NJ * 2 * 256], FP8,
                        kind="ExternalInput")
    outp = nc.dram_tensor("outp", [128, 128], FP32, kind="ExternalOutput")

    xt_v = xt.ap().rearrange("p (b j i n) -> p b j i n", b=BC, j=NJ, i=2)
    u_v = [u.ap().rearrange("p (cb j i c) -> p cb j i c", cb=2, j=NJ, i=2)
           for u in (u0, u1)]
    DR = mybir.MatmulPerfMode.DoubleRow

    # phase order = chunk consumption order on the sync ring
    PHASES = [(0, 0), (0, 1), (1, 0), (1, 1)]

    with tile.TileContext(nc) as tc:
        with (
            tc.tile_pool(name="xsb", bufs=1) as xpool,
            tc.tile_pool(name="usb", bufs=1) as upool,
            tc.tile_pool(name="esb", bufs=1) as epool,
            tc.tile_pool(name="single", bufs=1) as spool,
            tc.tile_pool(name="zp", bufs=1, space="PSUM") as zpool,
        ):
            # per-chunk tiles so a matmul only waits on its own chunk's DMA;
            # the first and last u chunks are split j-wise (128 KB) so the
            # stream starts earlier and the final chunk gates only 8 matmuls
            xb = [xpool.tile([128, 2, NJ, 2, 128], FP8, tag=f"x{bp}",
                             name=f"x{bp}") for bp in range(2)]
            uc = {}
            for nh, cb in PHASES:
                if (nh, cb) in ((0, 0), (1, 1)):
                    uc[(nh, cb)] = [
                        upool.tile([128, 2, 2, 256], FP8,
                                   tag=f"u{nh}_{cb}{h}", name=f"u{nh}_{cb}{h}")
                        for h in range(2)]
                else:
                    uc[(nh, cb)] = upool.tile(
                        [128, NJ, 2, 256], FP8,
                        tag=f"u{nh}_{cb}", name=f"u{nh}_{cb}")

            def uview(nh, cb, j):
                t = uc[(nh, cb)]
                if isinstance(t, list):
                    return t[j // 2][:, j % 2]
                return t[:, j]

            warm = spool.tile([128, 2, 384], FP8, tag="warm")
            nc.gpsimd.memset(warm[:], 0.0)

            # sync ring: u chunks in consumption order; scalar ring: x
            for nh, cb in PHASES:
                if isinstance(uc[(nh, cb)], list):
                    nc.sync.dma_start(uc[(nh, cb)][0][:],
                                      u_v[nh][:, cb, 0:2])
                    nc.sync.dma_start(uc[(nh, cb)][1][:],
                                      u_v[nh][:, cb, 2:4])
                else:
                    nc.sync.dma_start(uc[(nh, cb)][:], u_v[nh][:, cb])
            for bp in range(2):
                nc.scalar.dma_start(xb[bp][:], xt_v[:, 2 * bp:2 * bp + 2])

            # per-k sums land here slice-wise: group g = nh*4+bc covers
            # cols [g*16, (g+1)*16)
            ostage = spool.tile([128, 128], FP32, tag="os", name="ostage")

            # one PSUM bank per group: matmuls never wait on a reader
            zps = {(nh, bc): zpool.tile([128, 512], FP32, tag=f"zp{nh}{bc}",
                                        name=f"zp{nh}{bc}")
                   for nh in range(2) for bc in range(BC)}

            # warm-up: keep the PE busy while the first chunks stream in so
            # the p-state ramp clock starts early (writes bank (0,0); the
            # real group 0 resets it with start=True)
            for _ in range(WARM):
                nc.tensor.matmul(zps[(0, 0)][:, 0:256], warm[:, :, 0:128],
                                 warm[:, :, 128:384], start=True, stop=True,
                                 perf_mode=DR, skip_group_check=True)

            et = {(nh, bc): epool.tile([128, 512], BF16, tag=f"e{nh}{bc}",
                                       name=f"e{nh}{bc}")
                  for nh in range(2) for bc in range(BC)}

            for nh, cb in PHASES:
                sl = slice(cb * 256, (cb + 1) * 256)
                for bc in range(BC):
                    g = nh * 4 + bc
                    for j in range(NJ):
                        nc.tensor.matmul(
                            zps[(nh, bc)][:, sl], xb[bc // 2][:, bc % 2, j],
                            uview(nh, cb, j),
                            start=(j == 0), stop=(j == NJ - 1),
                            perf_mode=DR, skip_group_check=True)
                    # epilogue at priority 0: the scheduler places each op
                    # immediately after its producer, so the cross-engine
                    # semaphore targets stay minimal even though the cost
                    # model mispredicts matmul/DMA durations
                    with tc.high_priority():
                        nc.scalar.square(et[(nh, bc)][:, sl],
                                         zps[(nh, bc)][:, sl])
                        if nh == 0 and cb == 1:
                            # replicate-0 groups complete here: one 512-col
                            # subspace-sum per group
                            nc.vector.reduce_sum(
                                ostage[:, g * K:(g + 1) * K],
                                et[(nh, bc)].rearrange(
                                    "p (k c) -> p k c", c=d),
                                axis=mybir.AxisListType.X)
                            if bc == BC - 1:
                                nc.sync.dma_start(outp.ap()[:, 0:64],
                                                  ostage[:, 0:64])
                        elif nh == 1:
                            # replicate-1: per-half sums on DVE right after
                            # each square so the final chunk's tail is short
                            nc.vector.reduce_sum(
                                ostage[:, g * K + cb * 8:g * K + cb * 8 + 8],
                                et[(nh, bc)][:, sl].rearrange(
                                    "p (k c) -> p k c", c=d),
                                axis=mybir.AxisListType.X)
            with tc.high_priority():
                nc.sync.dma_start(outp.ap()[:, 64:128], ostage[:, 64:128])

    nc.compile()
    return nc


def _prep(x, Us):
    # fold chol(I - 0.5 U^T U) into U, then scale+quantize to fp8 e4m3
    Us64 = Us.astype(np.float64)
    G = np.einsum('skDa,skDb->skab', Us64, Us64)
    L = np.linalg.cholesky(np.eye(d)[None, None] - 0.5 * G)
    Ut = np.einsum('skDa,skab->skDb', Us64, L)                # (R,K,D,d)
    u8 = (Ut * SCALE).astype(np.float32).astype(ml_dtypes.float8_e4m3)
    x8 = np.ascontiguousarray(x.T).astype(ml_dtypes.float8_e4m3)  # (D, B)

    def u_img(r):  # one replicate -> [128, 2*NJ*2*256] (p, cb, j, i, c)
        ur = np.ascontiguousarray(u8[r].transpose(1, 0, 2)).reshape(D, K * d)
        return np.ascontiguousarray(
            ur.reshape(NJ, 2, 128, 2, 256).transpose(2, 3, 0, 1, 4)
        ).reshape(128, 2 * NJ * 2 * 256)

    def x_img(b4):  # one batch quarter -> [128, BC*NJ*2*128] (p, bc, j, i, n)
        xc = x8[:, NB * b4: NB * (b4 + 1)]                    # (D, 512)
        return np.ascontiguousarray(
            xc.reshape(NJ, 2, 128, BC, 128).transpose(2, 3, 0, 1, 4)
        ).reshape(128, BC * NJ * 2 * 128)

    u_imgs = [u_img(r) for r in range(R)]
    x_imgs = [x_img(b4) for b4 in range(BC)]
    in_maps = []
    for c in range(NCORES):
        s2, b4 = c // 4, c % 4
        in_maps.append({
            "xt": x_imgs[b4],
            "u0": u_imgs[2 * s2],
            "u1": u_imgs[2 * s2 + 1],
        })
    return in_maps


def kernel(x, Us, _trace=False):
    global LAST_RESULTS
    x = np.asarray(x)
    Us = np.asarray(Us)
    if "nc" not in _COMPILED:
        _COMPILED["nc"] = _build()
    nc = _COMPILED["nc"]
    in_maps = _prep(x, Us)
    res = run_bass_kernel_spmd(nc, in_maps, core_ids=list(range(NCORES)),
                               trace=_trace)
    LAST_RESULTS = res
    # base term: exact fp64 sum of squares on host (tiny vs the device work)
    base = 0.5 * float(np.sum(x.astype(np.float64) ** 2)) / B
    obj = np.empty(R, np.float32)
    for r in range(R):
        s2, nh = r // 2, r % 2
        # group col block = (nh*4 + bc)*16; z~ was scaled by SCALE
        terms = []
        for b4 in range(4):
            o = res.results[4 * s2 + b4]["outp"]              # [128, 128]
            for bc in range(BC):
                g = nh * 4 + bc
                terms.append(o[:, g * K:(g + 1) * K].astype(np.float64)
                             .max(axis=1))
        term = np.mean(np.stack(terms)) / (SCALE * SCALE)
        obj[r] = np.float32(base - term)
    return obj
